# revision 36
# baseline (speedup 1.0000x reference)
"""Bispectrum on S1xS1 — Trainium2 Bass kernel.

Full-input contract: kernel(x) with x (2, 64, 64) float32 returns
B (2, 4096, 4096) complex64 where, with X = fft2(x),
  B[b, (i,j), (p,q)] = X[b,i,j] * X[b,p,q] * conj(X[b,(i+p)%64,(j+q)%64]).

x is real, so X[-k,-l] = conj(X[k,l]) and B[rho(r), rho(c)] = conj(B[r,c])
with rho negating both frequency components. The device computes only rows
i in 0..33 (53% of the output); the host mirrors i in 34..63 by conjugation.

Sharding: each of the 8 cores computes ALL device rows for a 512-column
slice (p in [8k, 8k+8)) of both batches — an even split with no cross-core
communication. Per-core column offsets are folded into per-core DFT-matrix
inputs (spectrum row-rotated by 8k), so the SPMD program has no
core-dependent access patterns.

Per core:
  - 64-pt DFTs on PE via one host-passed packed DFT-matrix input (stage 2
    run twice: unrotated for the row/a-side, rotated+pre-scaled by 2^-5
    for the column/b-side + stack); both batches' setups are emitted
    before the main loops so batch-1 setup overlaps batch-0 compute
  - K=2 fp16 PE matmuls build U = outer-product components in PSUM (f32);
    ScalarE converts U -> fp16 SBUF with an extra 2^-1 scale
  - a sliding-window DMA over a doubled fp16 rotated spectrum builds the
    rolled-spectrum circulant stack C directly in fp16 in SBUF
  - the 4 real products of U * conj(C) and the re-add / im-sub all run as
    fp16 tensor ops at DVE 2x-mode (GpSimd is left idle: its SBUF port
    contention with DVE costs more than it contributes)
  - the fp16 [128, 1024] re|im plane chunk (still scaled 2^-11) is DMA'd
    contiguously; the host unscales by 2^11, interleaves the planes into
    complex64, and applies the Hermitian row mirror.
"""

import os
import sys

for _p in ("/opt/trn_rl_repo", "/opt/pypackages"):
    if _p not in sys.path:
        sys.path.insert(0, _p)

import numpy as np

M = 64
MN = M * M
NCORES = 8
NI = 34                 # i-values computed on device (0..33)
GL = NI // 2            # 17 row-pair blocks per batch
DEV_ROWS = NI * M       # 2176 rows per batch
COLS = MN // NCORES     # 512 columns per core
VSLOTS = 40             # circulant stack w-slots: v = 2*gl + pl <= 39
XDD_ROWS = VSLOTS + 1   # v + s <= 40

SC_C = 2.0 ** -5        # b-side/stack spectrum pre-scale (host, DFT mats)
SC_U = 2.0 ** -1        # extra scale at the U fp16 exit copy
IDSC = 2.0 ** 11        # identity-matmul scale undoing SC_C^2 * SC_U

_CACHE = {}


def _build_nc():
    import concourse.bass as bass
    import concourse.bacc as bacc
    import concourse.mybir as mybir
    from concourse.tile import TileContext

    f32 = mybir.dt.float32
    f16 = mybir.dt.float16
    nc = bacc.Bacc("TRN2")

    x = nc.declare_dram_parameter("x", [2, M, M], f32, isOutput=False)
    fmats = nc.declare_dram_parameter("fmats", [M, 6 * M], f32, isOutput=False)
    out = nc.declare_dram_parameter(
        "out", [2 * DEV_ROWS, 2 * COLS], f16, isOutput=True
    )

    # per-batch DRAM scratch
    dscratch = []
    for b in range(2):
        dscratch.append(
            dict(
                stk_d=nc.dram_tensor(f"stk_d{b}", [2, 2 * NI * M + 8 * M], f16),
                xddr=nc.dram_tensor(f"xddr{b}", [XDD_ROWS, 128], f16),
                xddi=nc.dram_tensor(f"xddi{b}", [XDD_ROWS, 128], f16),
            )
        )

    with TileContext(nc) as tc:
        with (
            tc.tile_pool(name="const", bufs=1) as cp,
            tc.tile_pool(name="big", bufs=1) as bp,
            tc.tile_pool(name="tmp", bufs=4) as tp,
            tc.tile_pool(name="chunkp", bufs=4) as kp,
        ):
          with tc.tile_pool(name="psum", bufs=2, space="PSUM") as pp:
              def sb64(src, tag):
                  t = cp.tile([M, M], f32, tag=tag)
                  nc.sync.dma_start(out=t, in_=src)
                  return t

              fm_sb = cp.tile([M, 6 * M], f32, tag="fmats")
              nc.sync.dma_start(out=fm_sb, in_=fmats[:, :])
              fr_sb = fm_sb[:, 0:M]
              fi_sb = fm_sb[:, M : 2 * M]
              fin_sb = fm_sb[:, 2 * M : 3 * M]
              frr_sb = fm_sb[:, 3 * M : 4 * M]
              fir_sb = fm_sb[:, 4 * M : 5 * M]
              finr_sb = fm_sb[:, 5 * M : 6 * M]

              def mm2(lhs1, rhs1, lhs2, rhs2_, tagn):
                  ps = pp.tile([M, M], f32, tag="fft")
                  nc.tensor.matmul(ps[:, :], lhsT=lhs1, rhs=rhs1, start=True, stop=False)
                  nc.tensor.matmul(ps[:, :], lhsT=lhs2, rhs=rhs2_, start=False, stop=True)
                  sb = cp.tile([M, M], f32, tag=tagn)
                  nc.scalar.copy(sb, ps)
                  return sb

              i32 = mybir.dt.int32
              f32r = mybir.dt.float32r
              MASK = -8192  # 0xFFFFE000: keep 10 explicit mantissa bits

              def setup(b):
                  d = dscratch[b]
                  x_sb = sb64(x[b, :, :], f"x{b}")
                  # x^T via 32x32 stream-transpose blocks
                  xt_sb = cp.tile([M, M], f32, tag=f"xt{b}")
                  for bi_ in range(2):
                      for bj in range(2):
                          nc.vector.transpose(
                              xt_sb[bi_ * 32 : bi_ * 32 + 32, bj * 32 : bj * 32 + 32],
                              x_sb[bj * 32 : bj * 32 + 32, bi_ * 32 : bi_ * 32 + 32],
                          )
                  # stage 1: W = x @ F
                  wr_ps = pp.tile([M, M], f32, tag="fft")
                  nc.tensor.matmul(
                      wr_ps[:, :], lhsT=xt_sb, rhs=fr_sb, start=True, stop=True
                  )
                  wr_sb = cp.tile([M, M], f32, tag=f"wr{b}")
                  nc.scalar.copy(wr_sb, wr_ps)
                  wi_ps = pp.tile([M, M], f32, tag="fft")
                  nc.tensor.matmul(
                      wi_ps[:, :], lhsT=xt_sb, rhs=fi_sb, start=True, stop=True
                  )
                  wi_sb = cp.tile([M, M], f32, tag=f"wi{b}")
                  nc.scalar.copy(wi_sb, wi_ps)

                  # stage 2 unrotated (a-side rows) and rotated (b-side + stack)
                  xr_sb = mm2(fr_sb, wr_sb, fin_sb, wi_sb, f"xr{b}")
                  xi_sb = mm2(fr_sb, wi_sb, fi_sb, wr_sb, f"xi{b}")
                  xrr_sb = mm2(frr_sb, wr_sb, finr_sb, wi_sb, f"xrr{b}")
                  xri_sb = mm2(frr_sb, wi_sb, fir_sb, wr_sb, f"xri{b}")

                  # fp16 spectrum copies for the circulant stack path
                  xrr16 = cp.tile([M, M], f16, tag=f"xrr16{b}")
                  nc.vector.tensor_copy(xrr16, xrr_sb)
                  xri16 = cp.tile([M, M], f16, tag=f"xri16{b}")
                  nc.vector.tensor_copy(xri16, xri_sb)

                  # doubled rotated spectrum (rows 0..XDD_ROWS all < 64: no wrap)
                  for (xdd, src_sb) in (
                      (d["xddr"], xrr16),
                      (d["xddi"], xri16),
                  ):
                      nc.scalar.dma_start(out=xdd[:, 0:64], in_=src_sb[0:XDD_ROWS, :])
                      nc.scalar.dma_start(out=xdd[:, 64:128], in_=src_sb[0:XDD_ROWS, :])

                  # circulant stack tiles; gathers are emitted separately (see
                  # gathers()) so their bulk can be deferred past the other
                  # batch's output DMAs in the HWDGE FIFOs
                  ch_r = bp.tile([128, VSLOTS * 64], f16, tag=f"ch_r{b}")
                  ch_i = bp.tile([128, VSLOTS * 64], f16, tag=f"ch_i{b}")
                  d["ch_r"], d["ch_i"] = ch_r, ch_i

                  # fp16 copies of the spectrum rows for K=2 fp16 U-matmuls
                  def to16(src_ap, rows, tagp, neg=False):
                      t16 = cp.tile([rows, M], f16, tag=tagp)
                      if neg:
                          nc.vector.tensor_scalar_mul(t16, src_ap, -1.0)
                      else:
                          nc.vector.tensor_copy(t16, src_ap)
                      return t16

                  ar16 = to16(xr_sb[0:NI, :], NI, f"ar{b}")
                  ai16 = to16(xi_sb[0:NI, :], NI, f"ai{b}")
                  ain16 = to16(xi_sb[0:NI, :], NI, f"ain{b}", neg=True)
                  br16 = to16(xrr_sb[0:8, :], 8, f"br{b}")
                  bi16 = to16(xri_sb[0:8, :], 8, f"bi{b}")

                  def stack_write(dst, rows_src, nrows, eng):
                      for r, t in enumerate(rows_src):
                          eng.dma_start(
                              out=dst[r : r + 1, :].rearrange(
                                  "r (p f) -> (r p) f", p=nrows
                              ),
                              in_=t,
                          )

                  # ur = ar (x) br + (-ai) (x) bi ; ui = ai (x) br + ar (x) bi
                  # all three operand stacks live in one DRAM scratch so a
                  # single read-back DMA restores them to SBUF
                  NM = NI * M
                  sd = d["stk_d"]

                  def stk_write(row, c0, src, nrows, eng):
                      n = nrows * M
                      eng.dma_start(
                          out=sd[row : row + 1, c0 : c0 + n].rearrange(
                              "r (p f) -> (r p) f", p=nrows
                          ),
                          in_=src,
                      )

                  # row-0 writes + readback on sync, row-1 writes on gpsimd
                  # (SWDGE), keeping scalar free for the xdd writes above
                  stk_write(0, 0, ar16, NI, nc.sync)
                  stk_write(1, 0, ain16, NI, nc.gpsimd)
                  stk_write(0, NM, ai16, NI, nc.sync)
                  stk_write(1, NM, ar16, NI, nc.gpsimd)
                  stk_write(0, 2 * NM, br16, 8, nc.sync)
                  stk_write(1, 2 * NM, bi16, 8, nc.gpsimd)
                  stk = bp.tile([2, 2 * NM + 8 * M], f16, tag=f"stk{b}")
                  nc.sync.dma_start(out=stk, in_=sd[:, :])
                  xa = stk[:, 0:NM]
                  xb = stk[:, NM : 2 * NM]
                  rhs2 = stk[:, 2 * NM : 2 * NM + 8 * M]

                  return dict(xa=xa, xb=xb, rhs2=rhs2, cr=ch_r, ci=ch_i)

              def gathers(b, lo, hi):
                  # ch[(s,j), (v,q)] = xdd[v+s, j+q] for v in [lo, hi)
                  d = dscratch[b]
                  call_engs = [nc.sync, nc.scalar, nc.scalar, nc.sync]
                  for ci_, (callt, xdd, s) in enumerate(
                      (c, xx, s)
                      for (c, xx) in ((d["ch_r"], d["xddr"]), (d["ch_i"], d["xddi"]))
                      for s in range(2)
                  ):
                      dest = callt[
                          s * 64 : (s + 1) * 64, lo * 64 : hi * 64
                      ].rearrange("j (v q) -> j v q", v=hi - lo)
                      srcap = bass.AP(
                          tensor=xdd,
                          offset=s * 128 + lo * 128,
                          ap=[[1, 64], [128, hi - lo], [1, 64]],
                      )
                      call_engs[ci_].dma_start(out=dest, in_=srcap)

              def mainloop(b, t_, gl_lo, gl_hi):
                  for gl in range(gl_lo, gl_hi):
                      v0 = 2 * gl
                      u2 = pp.tile([128, 2 * COLS], f32, tag="U2", bufs=3)
                      lsl = slice(gl * 128, gl * 128 + 128)
                      nc.tensor.matmul(
                          u2[:, 0:COLS],
                          lhsT=t_["xa"][:, lsl],
                          rhs=t_["rhs2"][:, :],
                          start=True, stop=True,
                      )
                      nc.tensor.matmul(
                          u2[:, COLS : 2 * COLS],
                          lhsT=t_["xb"][:, lsl],
                          rhs=t_["rhs2"][:, :],
                          start=True, stop=True,
                      )
                      # fp16 exit copy of U with extra 2^-1 scale
                      uh = kp.tile([128, 2 * COLS], f16, tag="uh")
                      nc.scalar.mul(uh, u2, SC_U)
                      urh = uh[:, 0:COLS]
                      uih = uh[:, COLS : 2 * COLS]

                      csl = slice(v0 * 64, v0 * 64 + COLS)
                      crs = t_["cr"][:, csl]
                      cis = t_["ci"][:, csl]
                      m1 = tp.tile([128, COLS], f16, tag="m1")
                      m2 = tp.tile([128, COLS], f16, tag="m2")
                      m3 = tp.tile([128, COLS], f16, tag="m3")
                      m4 = tp.tile([128, COLS], f16, tag="m4")
                      nc.vector.tensor_mul(m1, urh, crs)
                      nc.vector.tensor_mul(m2, uih, cis)
                      nc.vector.tensor_mul(m3, uih, crs)
                      nc.vector.tensor_mul(m4, urh, cis)

                      # re/im planes in fp16 (still scaled by 2^-11; host undoes)
                      chunk = kp.tile([128, 2 * COLS], f16, tag="chunk")
                      nc.vector.tensor_add(chunk[:, 0:COLS], m1, m2)
                      nc.vector.tensor_sub(chunk[:, COLS : 2 * COLS], m3, m4)

                      row0 = b * DEV_ROWS + gl * 128
                      out_eng = nc.sync if (gl % 2 == 0) else nc.scalar
                      out_eng.dma_start(
                          out=out[row0 : row0 + 128, :], in_=chunk[:, :]
                      )

              # Emission order tuned for the HWDGE per-engine FIFOs: batch-0
              # gets its full stack early; batch-1's bulk gather is deferred
              # into the middle of batch-0's main loop so batch-0's output
              # DMAs aren't head-of-line blocked behind it.
              t0 = setup(0)
              gathers(0, 0, 21)        # covers gl 0..6
              gathers(0, 21, VSLOTS)
              t1 = setup(1)
              mainloop(0, t0, 0, 6)
              gathers(1, 0, 21)
              mainloop(0, t0, 6, 12)
              gathers(1, 21, VSLOTS)
              mainloop(0, t0, 12, GL)
              mainloop(1, t1, 0, GL)
    nc.compile()
    return nc


def _dft_consts():
    k = np.arange(M)
    ang = -2.0 * np.pi * np.outer(k, k) / M
    Fr = np.cos(ang).astype(np.float32)
    Fi = np.sin(ang).astype(np.float32)
    return Fr, Fi


def _in_maps(x):
    Fr, Fi = _dft_consts()
    FiN = np.ascontiguousarray(-Fi)
    maps = []
    for core in range(NCORES):
        rFr = np.roll(Fr, -core * 8, axis=0) * SC_C
        rFi = np.roll(Fi, -core * 8, axis=0) * SC_C
        fmats = np.concatenate(
            [Fr, Fi, FiN, rFr.T, rFi.T, -rFi.T], axis=1
        ).astype(np.float32)
        maps.append({"x": x, "fmats": np.ascontiguousarray(fmats)})
    return maps


def _assemble(results):
    out = np.empty((2, MN, MN), dtype=np.complex64)
    for core in range(NCORES):
        blk = np.asarray(results[core]["out"])
        blk = blk.reshape(2, DEV_ROWS, 2 * COLS).astype(np.float32) * IDSC
        c0 = core * COLS
        out.real[:, 0:DEV_ROWS, c0 : c0 + COLS] = blk[:, :, 0:COLS]
        out.imag[:, 0:DEV_ROWS, c0 : c0 + COLS] = blk[:, :, COLS : 2 * COLS]
    # Hermitian mirror: rows i in 34..63 from conj at negated indices
    idx = np.arange(MN)
    rho = ((M - idx // M) % M) * M + (M - idx % M) % M
    rho_r = rho[DEV_ROWS:]
    for b in range(2):
        out[b, DEV_ROWS:, :] = np.conj(out[b, rho_r, :][:, rho])
    return out


def kernel(x):
    from concourse.bass_utils import run_bass_kernel_spmd

    x = np.asarray(x, dtype=np.float32)
    if "nc" not in _CACHE:
        _CACHE["nc"] = _build_nc()
    nc = _CACHE["nc"]
    trace = os.environ.get("BISPEC_TRACE", "0") == "1"
    res = run_bass_kernel_spmd(
        nc, _in_maps(x), core_ids=list(range(NCORES)), trace=trace
    )
    _CACHE["last_exec_time_ns"] = res.exec_time_ns
    _CACHE["last_res"] = res
    return _assemble(res.results)


# revision 38
# speedup vs baseline: 1.0638x; 1.0638x over previous
"""Bispectrum on S1xS1 — Trainium2 Bass kernel.

Full-input contract: kernel(x) with x (2, 64, 64) float32 returns
B (2, 4096, 4096) complex64 where, with X = fft2(x),
  B[b, (i,j), (p,q)] = X[b,i,j] * X[b,p,q] * conj(X[b,(i+p)%64,(j+q)%64]).

x is real, so X[-k,-l] = conj(X[k,l]) and B[rho(r), rho(c)] = conj(B[r,c])
with rho negating both frequency components. The device computes only rows
i in 0..33 (53% of the output); the host mirrors i in 34..63 by conjugation.

Sharding: each of the 8 cores computes ALL device rows for a 512-column
slice (p in [8k, 8k+8)) of both batches — an even split with no cross-core
communication. Per-core column offsets are folded into per-core DFT-matrix
inputs (spectrum row-rotated by 8k), so the SPMD program has no
core-dependent access patterns.

Per core:
  - 64-pt DFTs on PE via one host-passed packed DFT-matrix input (stage 2
    run twice: unrotated for the row/a-side, rotated+pre-scaled by 2^-5
    for the column/b-side + stack); both batches' setups are emitted
    before the main loops so batch-1 setup overlaps batch-0 compute
  - K=2 fp16 PE matmuls build U = outer-product components in PSUM (f32);
    ScalarE converts U -> fp16 SBUF with an extra 2^-1 scale
  - a sliding-window DMA over a doubled fp16 rotated spectrum builds the
    rolled-spectrum circulant stack C directly in fp16 in SBUF
  - the 4 real products of U * conj(C) and the re-add / im-sub all run as
    fp16 tensor ops at DVE 2x-mode (GpSimd is left idle: its SBUF port
    contention with DVE costs more than it contributes)
  - the fp16 [128, 1024] re|im plane chunk (still scaled 2^-11) is DMA'd
    contiguously; the host unscales by 2^11, interleaves the planes into
    complex64, and applies the Hermitian row mirror.
"""

import os
import sys

for _p in ("/opt/trn_rl_repo", "/opt/pypackages"):
    if _p not in sys.path:
        sys.path.insert(0, _p)

import numpy as np

M = 64
MN = M * M
NCORES = 8
NI = 34                 # i-values computed on device (0..33)
GL = NI // 2            # 17 row-pair blocks per batch
DEV_ROWS = NI * M       # 2176 rows per batch
COLS = MN // NCORES     # 512 columns per core
VSLOTS = 40             # circulant stack w-slots: v = 2*gl + pl <= 39
XDD_ROWS = VSLOTS + 1   # v + s <= 40

SC_C = 2.0 ** -5        # b-side/stack spectrum pre-scale (host, DFT mats)
SC_U = 2.0 ** -1        # extra scale at the U fp16 exit copy
IDSC = 2.0 ** 11        # identity-matmul scale undoing SC_C^2 * SC_U

_CACHE = {}


def _build_nc():
    import concourse.bass as bass
    import concourse.bacc as bacc
    import concourse.mybir as mybir
    from concourse.tile import TileContext

    f32 = mybir.dt.float32
    f16 = mybir.dt.float16
    nc = bacc.Bacc("TRN2")

    x = nc.declare_dram_parameter("x", [2, M, M], f32, isOutput=False)
    fmats = nc.declare_dram_parameter("fmats", [M, 6 * M], f32, isOutput=False)
    out = nc.declare_dram_parameter(
        "out", [2 * DEV_ROWS, 2 * COLS], f16, isOutput=True
    )

    # per-batch DRAM scratch
    dscratch = []
    for b in range(2):
        dscratch.append(
            dict(
                stk_d=nc.dram_tensor(f"stk_d{b}", [2, 2 * NI * M + 8 * M], f16),
                xddr=nc.dram_tensor(f"xddr{b}", [XDD_ROWS, 128], f16),
                xddi=nc.dram_tensor(f"xddi{b}", [XDD_ROWS, 128], f16),
            )
        )

    with TileContext(nc) as tc:
        with (
            tc.tile_pool(name="const", bufs=1) as cp,
            tc.tile_pool(name="big", bufs=1) as bp,
            tc.tile_pool(name="tmp", bufs=4) as tp,
            tc.tile_pool(name="chunkp", bufs=4) as kp,
        ):
          with tc.tile_pool(name="psum", bufs=2, space="PSUM") as pp:
              def sb64(src, tag):
                  t = cp.tile([M, M], f32, tag=tag)
                  nc.sync.dma_start(out=t, in_=src)
                  return t

              fm_sb = cp.tile([M, 6 * M], f32, tag="fmats")
              nc.sync.dma_start(out=fm_sb, in_=fmats[:, :])
              fr_sb = fm_sb[:, 0:M]
              fi_sb = fm_sb[:, M : 2 * M]
              fin_sb = fm_sb[:, 2 * M : 3 * M]
              frr_sb = fm_sb[:, 3 * M : 4 * M]
              fir_sb = fm_sb[:, 4 * M : 5 * M]
              finr_sb = fm_sb[:, 5 * M : 6 * M]

              def mm2(lhs1, rhs1, lhs2, rhs2_, tagn):
                  ps = pp.tile([M, M], f32, tag="fft")
                  nc.tensor.matmul(ps[:, :], lhsT=lhs1, rhs=rhs1, start=True, stop=False)
                  nc.tensor.matmul(ps[:, :], lhsT=lhs2, rhs=rhs2_, start=False, stop=True)
                  sb = cp.tile([M, M], f32, tag=tagn)
                  nc.scalar.copy(sb, ps)
                  return sb

              i32 = mybir.dt.int32
              f32r = mybir.dt.float32r
              MASK = -8192  # 0xFFFFE000: keep 10 explicit mantissa bits

              def setup(b):
                  d = dscratch[b]
                  x_sb = sb64(x[b, :, :], f"x{b}")
                  # x^T via 32x32 stream-transpose blocks
                  xt_sb = cp.tile([M, M], f32, tag=f"xt{b}")
                  for bi_ in range(2):
                      for bj in range(2):
                          nc.vector.transpose(
                              xt_sb[bi_ * 32 : bi_ * 32 + 32, bj * 32 : bj * 32 + 32],
                              x_sb[bj * 32 : bj * 32 + 32, bi_ * 32 : bi_ * 32 + 32],
                          )
                  # stage 1: W = x @ F
                  wr_ps = pp.tile([M, M], f32, tag="fft")
                  nc.tensor.matmul(
                      wr_ps[:, :], lhsT=xt_sb, rhs=fr_sb, start=True, stop=True
                  )
                  wr_sb = cp.tile([M, M], f32, tag=f"wr{b}")
                  nc.scalar.copy(wr_sb, wr_ps)
                  wi_ps = pp.tile([M, M], f32, tag="fft")
                  nc.tensor.matmul(
                      wi_ps[:, :], lhsT=xt_sb, rhs=fi_sb, start=True, stop=True
                  )
                  wi_sb = cp.tile([M, M], f32, tag=f"wi{b}")
                  nc.scalar.copy(wi_sb, wi_ps)

                  # stage 2 unrotated (a-side rows) and rotated (b-side + stack)
                  xr_sb = mm2(fr_sb, wr_sb, fin_sb, wi_sb, f"xr{b}")
                  xi_sb = mm2(fr_sb, wi_sb, fi_sb, wr_sb, f"xi{b}")
                  xrr_sb = mm2(frr_sb, wr_sb, finr_sb, wi_sb, f"xrr{b}")
                  xri_sb = mm2(frr_sb, wi_sb, fir_sb, wr_sb, f"xri{b}")

                  # fp16 spectrum copies for the circulant stack path
                  xrr16 = cp.tile([M, M], f16, tag=f"xrr16{b}")
                  nc.vector.tensor_copy(xrr16, xrr_sb)
                  xri16 = cp.tile([M, M], f16, tag=f"xri16{b}")
                  nc.vector.tensor_copy(xri16, xri_sb)

                  # doubled rotated spectrum (rows 0..XDD_ROWS all < 64: no wrap)
                  for (xdd, src_sb) in (
                      (d["xddr"], xrr16),
                      (d["xddi"], xri16),
                  ):
                      nc.scalar.dma_start(out=xdd[:, 0:64], in_=src_sb[0:XDD_ROWS, :])
                      nc.scalar.dma_start(out=xdd[:, 64:128], in_=src_sb[0:XDD_ROWS, :])

                  # circulant stack tiles; gathers are emitted separately (see
                  # gathers()) so their bulk can be deferred past the other
                  # batch's output DMAs in the HWDGE FIFOs
                  ch_r = bp.tile([128, VSLOTS * 64], f16, tag=f"ch_r{b}")
                  ch_i = bp.tile([128, VSLOTS * 64], f16, tag=f"ch_i{b}")
                  d["ch_r"], d["ch_i"] = ch_r, ch_i

                  # fp16 copies of the spectrum rows for K=2 fp16 U-matmuls
                  def to16(src_ap, rows, tagp, neg=False):
                      t16 = cp.tile([rows, M], f16, tag=tagp)
                      if neg:
                          nc.vector.tensor_scalar_mul(t16, src_ap, -1.0)
                      else:
                          nc.vector.tensor_copy(t16, src_ap)
                      return t16

                  ar16 = to16(xr_sb[0:NI, :], NI, f"ar{b}")
                  ai16 = to16(xi_sb[0:NI, :], NI, f"ai{b}")
                  ain16 = to16(xi_sb[0:NI, :], NI, f"ain{b}", neg=True)
                  br16 = to16(xrr_sb[0:8, :], 8, f"br{b}")
                  bi16 = to16(xri_sb[0:8, :], 8, f"bi{b}")

                  def stack_write(dst, rows_src, nrows, eng):
                      for r, t in enumerate(rows_src):
                          eng.dma_start(
                              out=dst[r : r + 1, :].rearrange(
                                  "r (p f) -> (r p) f", p=nrows
                              ),
                              in_=t,
                          )

                  # ur = ar (x) br + (-ai) (x) bi ; ui = ai (x) br + ar (x) bi
                  # all three operand stacks live in one DRAM scratch so a
                  # single read-back DMA restores them to SBUF
                  NM = NI * M
                  sd = d["stk_d"]

                  def stk_write(row, c0, src, nrows, eng):
                      n = nrows * M
                      eng.dma_start(
                          out=sd[row : row + 1, c0 : c0 + n].rearrange(
                              "r (p f) -> (r p) f", p=nrows
                          ),
                          in_=src,
                      )

                  # row-0 writes + readback on sync, row-1 writes on gpsimd
                  # (SWDGE), keeping scalar free for the xdd writes above
                  stk_write(0, 0, ar16, NI, nc.sync)
                  stk_write(1, 0, ain16, NI, nc.gpsimd)
                  stk_write(0, NM, ai16, NI, nc.sync)
                  stk_write(1, NM, ar16, NI, nc.gpsimd)
                  stk_write(0, 2 * NM, br16, 8, nc.sync)
                  stk_write(1, 2 * NM, bi16, 8, nc.gpsimd)
                  stk = bp.tile([2, 2 * NM + 8 * M], f16, tag=f"stk{b}")
                  nc.sync.dma_start(out=stk, in_=sd[:, :])
                  xa = stk[:, 0:NM]
                  xb = stk[:, NM : 2 * NM]
                  rhs2 = stk[:, 2 * NM : 2 * NM + 8 * M]

                  return dict(xa=xa, xb=xb, rhs2=rhs2, cr=ch_r, ci=ch_i)

              def gathers(b, lo, hi):
                  # ch[(s,j), (v,q)] = xdd[v+s, j+q] for v in [lo, hi)
                  d = dscratch[b]
                  call_engs = [nc.sync, nc.scalar, nc.scalar, nc.sync]
                  for ci_, (callt, xdd, s) in enumerate(
                      (c, xx, s)
                      for (c, xx) in ((d["ch_r"], d["xddr"]), (d["ch_i"], d["xddi"]))
                      for s in range(2)
                  ):
                      dest = callt[
                          s * 64 : (s + 1) * 64, lo * 64 : hi * 64
                      ].rearrange("j (v q) -> j v q", v=hi - lo)
                      srcap = bass.AP(
                          tensor=xdd,
                          offset=s * 128 + lo * 128,
                          ap=[[1, 64], [128, hi - lo], [1, 64]],
                      )
                      call_engs[ci_].dma_start(out=dest, in_=srcap)

              def mainloop(b, t_, gl_lo, gl_hi):
                  for gl in range(gl_lo, gl_hi):
                      v0 = 2 * gl
                      u2 = pp.tile([128, 2 * COLS], f32, tag="U2", bufs=3)
                      lsl = slice(gl * 128, gl * 128 + 128)
                      nc.tensor.matmul(
                          u2[:, 0:COLS],
                          lhsT=t_["xa"][:, lsl],
                          rhs=t_["rhs2"][:, :],
                          start=True, stop=True,
                      )
                      nc.tensor.matmul(
                          u2[:, COLS : 2 * COLS],
                          lhsT=t_["xb"][:, lsl],
                          rhs=t_["rhs2"][:, :],
                          start=True, stop=True,
                      )
                      # fp16 exit copy of U with extra 2^-1 scale
                      uh = kp.tile([128, 2 * COLS], f16, tag="uh")
                      nc.scalar.mul(uh, u2, SC_U)
                      urh = uh[:, 0:COLS]
                      uih = uh[:, COLS : 2 * COLS]

                      csl = slice(v0 * 64, v0 * 64 + COLS)
                      crs = t_["cr"][:, csl]
                      cis = t_["ci"][:, csl]
                      # fuse the 4 products into 2 double-width DVE ops:
                      # [ur|ui] (*) [cr|cr] -> [m1|m3], [ur|ui] (*) [ci|ci]
                      # -> [m4|m2], with the C window broadcast via a
                      # zero-stride middle dim
                      uv = uh[:, :].rearrange("p (g c) -> p g c", g=2)
                      pt = tp.tile([128, 4 * COLS], f16, tag="mp")
                      nc.vector.tensor_mul(
                          pt[:, 0 : 2 * COLS].rearrange("p (g c) -> p g c", g=2),
                          uv,
                          crs.unsqueeze(1).broadcast_to((128, 2, COLS)),
                      )
                      nc.vector.tensor_mul(
                          pt[:, 2 * COLS : 4 * COLS].rearrange(
                              "p (g c) -> p g c", g=2
                          ),
                          uv,
                          cis.unsqueeze(1).broadcast_to((128, 2, COLS)),
                      )
                      m1 = pt[:, 0:COLS]
                      m3 = pt[:, COLS : 2 * COLS]
                      m4 = pt[:, 2 * COLS : 3 * COLS]
                      m2 = pt[:, 3 * COLS : 4 * COLS]

                      # re/im planes in fp16 (still scaled by 2^-11; host undoes)
                      chunk = kp.tile([128, 2 * COLS], f16, tag="chunk")
                      nc.vector.tensor_add(chunk[:, 0:COLS], m1, m2)
                      nc.vector.tensor_sub(chunk[:, COLS : 2 * COLS], m3, m4)

                      row0 = b * DEV_ROWS + gl * 128
                      out_eng = nc.sync if (gl % 2 == 0) else nc.scalar
                      out_eng.dma_start(
                          out=out[row0 : row0 + 128, :], in_=chunk[:, :]
                      )

              # Emission order tuned for the HWDGE per-engine FIFOs: batch-0
              # gets its full stack early; batch-1's bulk gather is deferred
              # into the middle of batch-0's main loop so batch-0's output
              # DMAs aren't head-of-line blocked behind it.
              t0 = setup(0)
              gathers(0, 0, 9)         # covers gl 0 — loop starts ASAP
              gathers(0, 9, 21)        # covers gl 1..6
              gathers(0, 21, VSLOTS)
              t1 = setup(1)
              mainloop(0, t0, 0, 6)
              gathers(1, 0, 21)
              mainloop(0, t0, 6, 12)
              gathers(1, 21, VSLOTS)
              mainloop(0, t0, 12, GL)
              mainloop(1, t1, 0, GL)
    nc.compile()
    return nc


def _dft_consts():
    k = np.arange(M)
    ang = -2.0 * np.pi * np.outer(k, k) / M
    Fr = np.cos(ang).astype(np.float32)
    Fi = np.sin(ang).astype(np.float32)
    return Fr, Fi


def _in_maps(x):
    Fr, Fi = _dft_consts()
    FiN = np.ascontiguousarray(-Fi)
    maps = []
    for core in range(NCORES):
        rFr = np.roll(Fr, -core * 8, axis=0) * SC_C
        rFi = np.roll(Fi, -core * 8, axis=0) * SC_C
        fmats = np.concatenate(
            [Fr, Fi, FiN, rFr.T, rFi.T, -rFi.T], axis=1
        ).astype(np.float32)
        maps.append({"x": x, "fmats": np.ascontiguousarray(fmats)})
    return maps


def _assemble(results):
    out = np.empty((2, MN, MN), dtype=np.complex64)
    for core in range(NCORES):
        blk = np.asarray(results[core]["out"])
        blk = blk.reshape(2, DEV_ROWS, 2 * COLS).astype(np.float32) * IDSC
        c0 = core * COLS
        out.real[:, 0:DEV_ROWS, c0 : c0 + COLS] = blk[:, :, 0:COLS]
        out.imag[:, 0:DEV_ROWS, c0 : c0 + COLS] = blk[:, :, COLS : 2 * COLS]
    # Hermitian mirror: rows i in 34..63 from conj at negated indices
    idx = np.arange(MN)
    rho = ((M - idx // M) % M) * M + (M - idx % M) % M
    rho_r = rho[DEV_ROWS:]
    for b in range(2):
        out[b, DEV_ROWS:, :] = np.conj(out[b, rho_r, :][:, rho])
    return out


def kernel(x):
    from concourse.bass_utils import run_bass_kernel_spmd

    x = np.asarray(x, dtype=np.float32)
    if "nc" not in _CACHE:
        _CACHE["nc"] = _build_nc()
    nc = _CACHE["nc"]
    trace = os.environ.get("BISPEC_TRACE", "0") == "1"
    res = run_bass_kernel_spmd(
        nc, _in_maps(x), core_ids=list(range(NCORES)), trace=trace
    )
    _CACHE["last_exec_time_ns"] = res.exec_time_ns
    _CACHE["last_res"] = res
    return _assemble(res.results)


# revision 42
# speedup vs baseline: 1.0836x; 1.0186x over previous
"""Bispectrum on S1xS1 — Trainium2 Bass kernel.

Full-input contract: kernel(x) with x (2, 64, 64) float32 returns
B (2, 4096, 4096) complex64 where, with X = fft2(x),
  B[b, (i,j), (p,q)] = X[b,i,j] * X[b,p,q] * conj(X[b,(i+p)%64,(j+q)%64]).

x is real, so X[-k,-l] = conj(X[k,l]) and B[rho(r), rho(c)] = conj(B[r,c])
with rho negating both frequency components. The device computes only rows
i in 0..33 (53% of the output); the host mirrors i in 34..63 by conjugation.

Sharding: each of the 8 cores computes ALL device rows for a 512-column
slice (p in [8k, 8k+8)) of both batches — an even split with no cross-core
communication. Per-core column offsets are folded into per-core DFT-matrix
inputs (spectrum row-rotated by 8k), so the SPMD program has no
core-dependent access patterns.

Per core:
  - 64-pt DFTs on PE via one host-passed packed DFT-matrix input (stage 2
    run twice: unrotated for the row/a-side, rotated+pre-scaled by 2^-5
    for the column/b-side + stack); both batches' setups are emitted
    before the main loops so batch-1 setup overlaps batch-0 compute
  - K=2 fp16 PE matmuls build U = outer-product components in PSUM (f32);
    ScalarE converts U -> fp16 SBUF with an extra 2^-1 scale
  - a sliding-window DMA over a doubled fp16 rotated spectrum builds the
    rolled-spectrum circulant stack C directly in fp16 in SBUF
  - the 4 real products of U * conj(C) and the re-add / im-sub all run as
    fp16 tensor ops at DVE 2x-mode (GpSimd is left idle: its SBUF port
    contention with DVE costs more than it contributes)
  - the fp16 [128, 1024] re|im plane chunk (still scaled 2^-11) is DMA'd
    contiguously; the host unscales by 2^11, interleaves the planes into
    complex64, and applies the Hermitian row mirror.
"""

import os
import sys

for _p in ("/opt/trn_rl_repo", "/opt/pypackages"):
    if _p not in sys.path:
        sys.path.insert(0, _p)

import numpy as np

M = 64
MN = M * M
NCORES = 8
NI = 34                 # i-values computed on device (0..33)
GL = NI // 2            # 17 row-pair blocks per batch
DEV_ROWS = NI * M       # 2176 rows per batch
COLS = MN // NCORES     # 512 columns per core
VSLOTS = 40             # circulant stack w-slots: v = 2*gl + pl <= 39
XDD_ROWS = VSLOTS + 1   # v + s <= 40

SC_C = 2.0 ** -5        # b-side/stack spectrum pre-scale (host, DFT mats)
SC_U = 2.0 ** -1        # extra scale at the U fp16 exit copy
IDSC = 2.0 ** 11        # identity-matmul scale undoing SC_C^2 * SC_U

_CACHE = {}


def _build_nc():
    import concourse.bass as bass
    import concourse.bacc as bacc
    import concourse.mybir as mybir
    from concourse.tile import TileContext

    f32 = mybir.dt.float32
    f16 = mybir.dt.float16
    nc = bacc.Bacc("TRN2")

    x = nc.declare_dram_parameter("x", [2, M, M], f32, isOutput=False)
    fmats = nc.declare_dram_parameter("fmats", [M, 6 * M], f32, isOutput=False)
    out = nc.declare_dram_parameter(
        "out", [2 * DEV_ROWS, 2 * COLS], f16, isOutput=True
    )

    # per-batch DRAM scratch
    dscratch = []
    for b in range(2):
        dscratch.append(
            dict(
                stk_d=nc.dram_tensor(f"stk_d{b}", [2, 2 * NI * M + 8 * M], f16),
                xddr=nc.dram_tensor(f"xddr{b}", [XDD_ROWS, 128], f16),
                xddi=nc.dram_tensor(f"xddi{b}", [XDD_ROWS, 128], f16),
            )
        )

    with TileContext(nc) as tc:
        with (
            tc.tile_pool(name="const", bufs=1) as cp,
            tc.tile_pool(name="big", bufs=1) as bp,
            tc.tile_pool(name="tmp", bufs=4) as tp,
            tc.tile_pool(name="chunkp", bufs=4) as kp,
        ):
          with tc.tile_pool(name="psum", bufs=2, space="PSUM") as pp:
              def sb64(src, tag):
                  t = cp.tile([M, M], f32, tag=tag)
                  nc.sync.dma_start(out=t, in_=src)
                  return t

              fm_sb = cp.tile([M, 6 * M], f32, tag="fmats")
              nc.sync.dma_start(out=fm_sb, in_=fmats[:, :])
              fr_sb = fm_sb[:, 0:M]
              fi_sb = fm_sb[:, M : 2 * M]
              fin_sb = fm_sb[:, 2 * M : 3 * M]
              frr_sb = fm_sb[:, 3 * M : 4 * M]
              fir_sb = fm_sb[:, 4 * M : 5 * M]
              finr_sb = fm_sb[:, 5 * M : 6 * M]

              def mm2(lhs1, rhs1, lhs2, rhs2_, tagn):
                  ps = pp.tile([M, M], f32, tag="fft")
                  nc.tensor.matmul(ps[:, :], lhsT=lhs1, rhs=rhs1, start=True, stop=False)
                  nc.tensor.matmul(ps[:, :], lhsT=lhs2, rhs=rhs2_, start=False, stop=True)
                  sb = cp.tile([M, M], f32, tag=tagn)
                  nc.scalar.copy(sb, ps)
                  return sb

              i32 = mybir.dt.int32
              f32r = mybir.dt.float32r
              MASK = -8192  # 0xFFFFE000: keep 10 explicit mantissa bits

              def setup(b):
                  d = dscratch[b]
                  x_sb = sb64(x[b, :, :], f"x{b}")
                  # x^T via 32x32 stream-transpose blocks
                  xt_sb = cp.tile([M, M], f32, tag=f"xt{b}")
                  for bi_ in range(2):
                      for bj in range(2):
                          nc.vector.transpose(
                              xt_sb[bi_ * 32 : bi_ * 32 + 32, bj * 32 : bj * 32 + 32],
                              x_sb[bj * 32 : bj * 32 + 32, bi_ * 32 : bi_ * 32 + 32],
                          )
                  # stage 1: W = x @ F
                  wr_ps = pp.tile([M, M], f32, tag="fft")
                  nc.tensor.matmul(
                      wr_ps[:, :], lhsT=xt_sb, rhs=fr_sb, start=True, stop=True
                  )
                  wr_sb = cp.tile([M, M], f32, tag=f"wr{b}")
                  nc.scalar.copy(wr_sb, wr_ps)
                  wi_ps = pp.tile([M, M], f32, tag="fft")
                  nc.tensor.matmul(
                      wi_ps[:, :], lhsT=xt_sb, rhs=fi_sb, start=True, stop=True
                  )
                  wi_sb = cp.tile([M, M], f32, tag=f"wi{b}")
                  nc.scalar.copy(wi_sb, wi_ps)

                  # stage 2 unrotated (a-side rows) and rotated (b-side + stack)
                  xr_sb = mm2(fr_sb, wr_sb, fin_sb, wi_sb, f"xr{b}")
                  xi_sb = mm2(fr_sb, wi_sb, fi_sb, wr_sb, f"xi{b}")
                  xrr_sb = mm2(frr_sb, wr_sb, finr_sb, wi_sb, f"xrr{b}")
                  xri_sb = mm2(frr_sb, wi_sb, fir_sb, wr_sb, f"xri{b}")

                  # fp16 spectrum copies for the circulant stack path
                  xrr16 = cp.tile([M, M], f16, tag=f"xrr16{b}")
                  nc.vector.tensor_copy(xrr16, xrr_sb)
                  xri16 = cp.tile([M, M], f16, tag=f"xri16{b}")
                  nc.vector.tensor_copy(xri16, xri_sb)

                  # doubled rotated spectrum (rows 0..XDD_ROWS all < 64: no wrap)
                  for (xdd, src_sb) in (
                      (d["xddr"], xrr16),
                      (d["xddi"], xri16),
                  ):
                      nc.scalar.dma_start(out=xdd[:, 0:64], in_=src_sb[0:XDD_ROWS, :])
                      nc.scalar.dma_start(out=xdd[:, 64:128], in_=src_sb[0:XDD_ROWS, :])

                  # circulant stacks live in ONE tile [cr-full | ci-full] so a
                  # (cr,ci) window pair is a single affine AP; gathers are
                  # emitted separately (see gathers()) so their bulk can be
                  # deferred past the other batch's output DMAs in the HWDGE
                  # FIFOs
                  ch = bp.tile([128, 2 * VSLOTS * 64], f16, tag=f"ch{b}")
                  d["ch"] = ch
                  d["ch_r"] = ch[:, 0 : VSLOTS * 64]
                  d["ch_i"] = ch[:, VSLOTS * 64 : 2 * VSLOTS * 64]

                  # fp16 copies of the spectrum rows for K=2 fp16 U-matmuls
                  def to16(src_ap, rows, tagp, neg=False):
                      t16 = cp.tile([rows, M], f16, tag=tagp)
                      if neg:
                          nc.vector.tensor_scalar_mul(t16, src_ap, -1.0)
                      else:
                          nc.vector.tensor_copy(t16, src_ap)
                      return t16

                  ar16 = to16(xr_sb[0:NI, :], NI, f"ar{b}")
                  ai16 = to16(xi_sb[0:NI, :], NI, f"ai{b}")
                  ain16 = to16(xi_sb[0:NI, :], NI, f"ain{b}", neg=True)
                  br16 = to16(xrr_sb[0:8, :], 8, f"br{b}")
                  bi16 = to16(xri_sb[0:8, :], 8, f"bi{b}")

                  def stack_write(dst, rows_src, nrows, eng):
                      for r, t in enumerate(rows_src):
                          eng.dma_start(
                              out=dst[r : r + 1, :].rearrange(
                                  "r (p f) -> (r p) f", p=nrows
                              ),
                              in_=t,
                          )

                  # ur = ar (x) br + (-ai) (x) bi ; ui = ai (x) br + ar (x) bi
                  # all three operand stacks live in one DRAM scratch so a
                  # single read-back DMA restores them to SBUF
                  NM = NI * M
                  sd = d["stk_d"]

                  def stk_write(row, c0, src, nrows, eng):
                      n = nrows * M
                      eng.dma_start(
                          out=sd[row : row + 1, c0 : c0 + n].rearrange(
                              "r (p f) -> (r p) f", p=nrows
                          ),
                          in_=src,
                      )

                  # row-0 writes + readback on sync, row-1 writes on gpsimd
                  # (SWDGE), keeping scalar free for the xdd writes above
                  stk_write(0, 0, ar16, NI, nc.sync)
                  stk_write(1, 0, ain16, NI, nc.gpsimd)
                  stk_write(0, NM, ai16, NI, nc.sync)
                  stk_write(1, NM, ar16, NI, nc.gpsimd)
                  stk_write(0, 2 * NM, br16, 8, nc.sync)
                  stk_write(1, 2 * NM, bi16, 8, nc.gpsimd)
                  stk = bp.tile([2, 2 * NM + 8 * M], f16, tag=f"stk{b}")
                  nc.sync.dma_start(out=stk, in_=sd[:, :])
                  xa = stk[:, 0:NM]
                  xb = stk[:, NM : 2 * NM]
                  rhs2 = stk[:, 2 * NM : 2 * NM + 8 * M]

                  return dict(xa=xa, xb=xb, rhs2=rhs2, ch=ch)

              def gathers(b, lo, hi):
                  # ch[(s,j), (v,q)] = xdd[v+s, j+q] for v in [lo, hi)
                  d = dscratch[b]
                  call_engs = [nc.sync, nc.scalar, nc.scalar, nc.sync]
                  for ci_, (callt, xdd, s) in enumerate(
                      (c, xx, s)
                      for (c, xx) in ((d["ch_r"], d["xddr"]), (d["ch_i"], d["xddi"]))
                      for s in range(2)
                  ):
                      dest = callt[
                          s * 64 : (s + 1) * 64, lo * 64 : hi * 64
                      ].rearrange("j (v q) -> j v q", v=hi - lo)
                      srcap = bass.AP(
                          tensor=xdd,
                          offset=s * 128 + lo * 128,
                          ap=[[1, 64], [128, hi - lo], [1, 64]],
                      )
                      call_engs[ci_].dma_start(out=dest, in_=srcap)

              def mainloop(b, t_, gl_lo, gl_hi):
                  for gl in range(gl_lo, gl_hi):
                      v0 = 2 * gl
                      u2 = pp.tile([128, 2 * COLS], f32, tag="U2", bufs=3)
                      lsl = slice(gl * 128, gl * 128 + 128)
                      nc.tensor.matmul(
                          u2[:, 0:COLS],
                          lhsT=t_["xa"][:, lsl],
                          rhs=t_["rhs2"][:, :],
                          start=True, stop=True,
                      )
                      nc.tensor.matmul(
                          u2[:, COLS : 2 * COLS],
                          lhsT=t_["xb"][:, lsl],
                          rhs=t_["rhs2"][:, :],
                          start=True, stop=True,
                      )
                      # fp16 exit copy of U with extra 2^-1 scale
                      uh = kp.tile([128, 2 * COLS], f16, tag="uh")
                      nc.scalar.mul(uh, u2, SC_U)
                      urh = uh[:, 0:COLS]
                      uih = uh[:, COLS : 2 * COLS]

                      # all 4 products in ONE quad-width DVE op:
                      # [ur ui ur ui] (*) [cr cr ci ci] -> [m1 m3 m4 m2],
                      # broadcasting uh over the (cr,ci) dim and the C window
                      # pair over the (ur,ui) dim with zero-stride AP dims
                      in0 = (
                          uh[:, :]
                          .rearrange("p (h c) -> p h c", h=2)
                          .unsqueeze(1)
                          .broadcast_to((128, 2, 2, COLS))
                      )
                      in1 = (
                          t_["ch"]
                          .rearrange("p (g q) -> p g q", g=2)[
                              :, :, v0 * 64 : v0 * 64 + COLS
                          ]
                          .unsqueeze(2)
                          .broadcast_to((128, 2, 2, COLS))
                      )
                      pt = tp.tile([128, 4 * COLS], f16, tag="mp")
                      nc.vector.tensor_mul(
                          pt[:, :].rearrange("p (g h c) -> p g h c", g=2, h=2),
                          in0,
                          in1,
                      )
                      m1 = pt[:, 0:COLS]
                      m3 = pt[:, COLS : 2 * COLS]
                      m4 = pt[:, 2 * COLS : 3 * COLS]
                      m2 = pt[:, 3 * COLS : 4 * COLS]

                      # re/im planes in fp16 (still scaled by 2^-11; host undoes)
                      chunk = kp.tile([128, 2 * COLS], f16, tag="chunk")
                      nc.vector.tensor_add(chunk[:, 0:COLS], m1, m2)
                      nc.vector.tensor_sub(chunk[:, COLS : 2 * COLS], m3, m4)

                      row0 = b * DEV_ROWS + gl * 128
                      out_eng = nc.sync if (gl % 2 == 0) else nc.scalar
                      out_eng.dma_start(
                          out=out[row0 : row0 + 128, :], in_=chunk[:, :]
                      )

              # Emission order tuned for the HWDGE per-engine FIFOs: batch-0
              # gets its full stack early; batch-1's bulk gather is deferred
              # into the middle of batch-0's main loop so batch-0's output
              # DMAs aren't head-of-line blocked behind it.
              t0 = setup(0)
              gathers(0, 0, 9)         # covers gl 0 — loop starts ASAP
              gathers(0, 9, 21)        # covers gl 1..6
              gathers(0, 21, VSLOTS)
              t1 = setup(1)
              mainloop(0, t0, 0, 8)
              gathers(1, 0, 21)
              mainloop(0, t0, 8, 14)
              gathers(1, 21, VSLOTS)
              mainloop(0, t0, 14, GL)
              mainloop(1, t1, 0, GL)
    nc.compile()
    return nc


def _dft_consts():
    k = np.arange(M)
    ang = -2.0 * np.pi * np.outer(k, k) / M
    Fr = np.cos(ang).astype(np.float32)
    Fi = np.sin(ang).astype(np.float32)
    return Fr, Fi


def _in_maps(x):
    Fr, Fi = _dft_consts()
    FiN = np.ascontiguousarray(-Fi)
    maps = []
    for core in range(NCORES):
        rFr = np.roll(Fr, -core * 8, axis=0) * SC_C
        rFi = np.roll(Fi, -core * 8, axis=0) * SC_C
        fmats = np.concatenate(
            [Fr, Fi, FiN, rFr.T, rFi.T, -rFi.T], axis=1
        ).astype(np.float32)
        maps.append({"x": x, "fmats": np.ascontiguousarray(fmats)})
    return maps


def _assemble(results):
    out = np.empty((2, MN, MN), dtype=np.complex64)
    for core in range(NCORES):
        blk = np.asarray(results[core]["out"])
        blk = blk.reshape(2, DEV_ROWS, 2 * COLS).astype(np.float32) * IDSC
        c0 = core * COLS
        out.real[:, 0:DEV_ROWS, c0 : c0 + COLS] = blk[:, :, 0:COLS]
        out.imag[:, 0:DEV_ROWS, c0 : c0 + COLS] = blk[:, :, COLS : 2 * COLS]
    # Hermitian mirror: rows i in 34..63 from conj at negated indices
    idx = np.arange(MN)
    rho = ((M - idx // M) % M) * M + (M - idx % M) % M
    rho_r = rho[DEV_ROWS:]
    for b in range(2):
        out[b, DEV_ROWS:, :] = np.conj(out[b, rho_r, :][:, rho])
    return out


def kernel(x):
    from concourse.bass_utils import run_bass_kernel_spmd

    x = np.asarray(x, dtype=np.float32)
    if "nc" not in _CACHE:
        _CACHE["nc"] = _build_nc()
    nc = _CACHE["nc"]
    trace = os.environ.get("BISPEC_TRACE", "0") == "1"
    res = run_bass_kernel_spmd(
        nc, _in_maps(x), core_ids=list(range(NCORES)), trace=trace
    )
    _CACHE["last_exec_time_ns"] = res.exec_time_ns
    _CACHE["last_res"] = res
    return _assemble(res.results)
